# revision 41
# baseline (speedup 1.0000x reference)
"""Trainium2 Bass kernel for nn_MoEElementFusion (moe_routing).

Strategy (8 NeuronCores, SPMD, two launches with host routing in between):
  Phase 1 (token-data-parallel): each core takes 1/8 of the 8192 (view,token)
  columns, streamed in 512-column chunks (chunk-major DRAM layouts so every
  DMA is contiguous with 4KB lines):
      h  = x @ proj_w + proj_b     (fp16 on the PE, psum fp32)
      r  = h @ router_w
      d2 = |r|^2 - 2 r.keys^T + |keys|^2
  h^T chunks are DMA'd out as they finish (overlapped with router compute).

  Host: tokens whose 4th/5th logit gap is under REPAIR_MARGIN get their d2
  row recomputed exactly in fp32; then logits, stable top-4, softmax gates.
  A segment plan packs each expert's selected tokens (128-col blocks) into a
  per-core-uniform pattern of segments (e.g. [16,8,4,2,2,1] blocks); each
  segment carries its own expert weights, so the padded column count per
  core is ~4224 instead of 9*512=4608 and weight DMA drops to ~6 loads.

  Phase 2 (compiled at runtime once the pattern is known): per segment, FFN
  in fp16 over 512/256/128-column chunks:
      out^T = (w2^T-mm(gelu(w1^T-mm(h^T) + b1)) + b2) * gates
  weights host-packed in the exact SBUF layout (contiguous, 4 m-tile groups
  interleaved w1/w2 for streaming), triple buffered; outputs stored fp16.

  Host combine: fused[:, tok] += out columns per segment; sum the two views.
"""

import math
import os

import numpy as np

import concourse.bass as bass
import concourse.bacc as bacc
import concourse.mybir as mybir
import concourse.tile as tile
from concourse.bass_utils import run_bass_kernel_spmd

# Problem dims (hardcoded per spec)
V, B, T, D, E, K = 2, 4, 1024, 512, 16, 4
H = 4 * D
N = B * T          # tokens per view
NT = V * N         # total (view, token) columns = 8192
NC = 8             # cores
PC = NT // NC      # phase-1 columns per core = 1024
BLK = 128          # phase-2 packing block (columns)

F32 = mybir.dt.float32
F16 = mybir.dt.float16
AF = mybir.ActivationFunctionType
ALU = mybir.AluOpType

DK = D // 128      # 4 k-tiles over D
HK = H // 128      # 16 k-tiles over H

CH = 512           # phase-1 column chunk
NCH = PC // CH     # 2 chunks

REPAIR_MARGIN = 0.02
MG = 2             # phase-2 weight DMA group: m-tiles per DMA

# Filled by kernel() for test harness introspection.
last_stats: dict = {}


# --------------------------------------------------------------------------
# Phase 1: h = x@pw + pb ; r = h@rw ; d2 = |r|^2 - 2 r.k + |k|^2
# --------------------------------------------------------------------------
def _phase1_nc() -> bass.Bass:
    nc = bacc.Bacc("TRN2", target_bir_lowering=False, num_devices=NC)
    # Two 1MB input blobs so the critical path is 1-2 large DMAs:
    #   blob n = [ w (2048 cols) | xT chunk n (2048 cols) ]  fp16
    # w packed [p, m*512 + k*128 + j] = w[k*128+p, m*128+j]; blob0 carries
    # pw, blob1 carries prw = proj_w @ router_w (r is computed directly
    # from x and never waits on the h activation writes).
    # xT chunk [p, k*CH+c] = x^T[k*128+p, n*CH+c]
    blob = nc.dram_tensor("blob", [128, NCH, 2 * DK * CH], F16, kind="ExternalInput")
    pb = nc.dram_tensor("pb", [128, DK], F32, kind="ExternalInput")
    prb = nc.dram_tensor("prb", [128, DK], F32, kind="ExternalInput")
    kT2 = nc.dram_tensor("kT2", [128, DK, E], F16, kind="ExternalInput")
    kk1 = nc.dram_tensor("kk1", [1, E], F32, kind="ExternalInput")
    onc = nc.dram_tensor("onc", [128, 1], F16, kind="ExternalInput")
    onr = nc.dram_tensor("onr", [1, 512], F32, kind="ExternalInput")
    hT = nc.dram_tensor("hT", [128, NCH, DK * CH], F16, kind="ExternalOutput")
    d2T = nc.dram_tensor("d2T", [E, PC], F32, kind="ExternalOutput")

    with tile.TileContext(nc) as tc:
        with (
            tc.tile_pool(name="const", bufs=1) as cpool,
            tc.tile_pool(name="act", bufs=1) as apool,
            tc.tile_pool(name="rt", bufs=2) as rtpool,
            tc.tile_pool(name="r2p", bufs=2) as r2pool,
            tc.tile_pool(name="ps", bufs=2, space="PSUM") as pspool,
            tc.tile_pool(name="ps_small", bufs=2, space="PSUM") as psmall,
        ):
            blob_sb = cpool.tile([128, NCH, 2 * DK * CH], F16, tag="blob")
            # inputs on the ACT HWDGE queue, critical path first:
            # blob0 = pw + x chunk 0, then the small constants, then blob1
            nc.scalar.dma_start(blob_sb[:, 0, :], blob[:, 0, :])
            pb_sb = cpool.tile([128, DK], F32, tag="pb")
            nc.scalar.dma_start(pb_sb[:], pb[:])
            prb_sb = cpool.tile([128, DK], F32, tag="prb")
            nc.scalar.dma_start(prb_sb[:], prb[:])
            nc.scalar.dma_start(blob_sb[:, 1, :], blob[:, 1, :])
            kT2_sb = cpool.tile([128, DK, E], F16, tag="kT2")
            nc.scalar.dma_start(kT2_sb[:], kT2[:])
            kk_sb = cpool.tile([1, E], F32, tag="kk")
            nc.scalar.dma_start(kk_sb[:], kk1[:])
            onc_sb = cpool.tile([128, 1], F16, tag="onc")
            nc.scalar.dma_start(onc_sb[:], onc[:])
            onr_sb = cpool.tile([1, 512], F32, tag="onr")
            nc.scalar.dma_start(onr_sb[:], onr[:])

            XO = DK * CH  # xT offset within a blob

            hT_sb = apool.tile([128, NCH, DK * CH], F16, tag="hT")
            rr_sb = apool.tile([1, PC], F32, tag="rr")
            d2_sb = apool.tile([E, PC], F32, tag="d2")

            # HAM warm-up: junk matmuls on zeroed SBUF spanning the input
            # DMA wait, so the real matmuls start at full clock.
            warm = cpool.tile([128, 128], F16, tag="warm")
            warm2 = cpool.tile([128, 512], F16, tag="warm2")
            nc.vector.memset(warm[:], 0.0)
            nc.vector.memset(warm2[:], 0.0)
            for _ in range(13):
                wps = pspool.tile([128, CH], F32, tag="ps")
                nc.tensor.matmul(wps[:], warm[:], warm2[:], start=True, stop=True)

            for n in range(NCH):
                sl = slice(n * CH, (n + 1) * CH)
                # h^T chunk
                for m in range(DK):
                    ps = pspool.tile([128, CH], F32, tag="ps")
                    for k in range(DK):
                        nc.tensor.matmul(
                            ps[:],
                            blob_sb[:, 0, m * 512 + k * 128 : m * 512 + (k + 1) * 128],
                            blob_sb[:, n, XO + k * CH : XO + (k + 1) * CH],
                            start=(k == 0),
                            stop=(k == DK - 1),
                        )
                    nc.scalar.activation(
                        hT_sb[:, n, m * CH : (m + 1) * CH], ps[:],
                        AF.Identity, bias=pb_sb[:, m : m + 1],
                    )
                    nc.sync.dma_start(
                        hT[:, n, m * CH : (m + 1) * CH],
                        hT_sb[:, n, m * CH : (m + 1) * CH],
                    )
                # r^T chunk (from x directly, via prw)
                rt = rtpool.tile([128, DK, CH], F16, tag="rt")
                for m in range(DK):
                    ps = pspool.tile([128, CH], F32, tag="ps")
                    for k in range(DK):
                        nc.tensor.matmul(
                            ps[:],
                            blob_sb[:, 1, m * 512 + k * 128 : m * 512 + (k + 1) * 128],
                            blob_sb[:, n, XO + k * CH : XO + (k + 1) * CH],
                            start=(k == 0),
                            stop=(k == DK - 1),
                        )
                    nc.scalar.activation(
                        rt[:, m, :], ps[:], AF.Identity, bias=prb_sb[:, m : m + 1]
                    )
                # rr = sum_d r^2
                r2_sb = r2pool.tile([128, DK, CH], F16, tag="r2")
                for k in range(DK):
                    nc.vector.tensor_mul(r2_sb[:, k, :], rt[:, k, :], rt[:, k, :])
                ps1 = psmall.tile([1, CH], F32, tag="ps1")
                for k in range(DK):
                    nc.tensor.matmul(
                        ps1[:], onc_sb[:], r2_sb[:, k, :],
                        start=(k == 0), stop=(k == DK - 1),
                    )
                nc.scalar.activation(rr_sb[:, sl], ps1[:], AF.Copy)
                # d2 = (-2 keys).r + |k|^2 + rr
                psA = psmall.tile([E, CH], F32, tag="psA")
                for k in range(DK):
                    nc.tensor.matmul(
                        psA[:], kT2_sb[:, k, :], rt[:, k, :],
                        start=(k == 0), stop=False,
                    )
                nc.tensor.matmul(psA[:], kk_sb[:], onr_sb[:], start=False, stop=False)
                nc.tensor.matmul(
                    psA[:], onr_sb[:, 0:E], rr_sb[:, sl], start=False, stop=True
                )
                nc.scalar.activation(d2_sb[:, sl], psA[:], AF.Copy)
                nc.sync.dma_start(d2T[:, sl], d2_sb[:, sl])
    nc.compile()
    return nc


# --------------------------------------------------------------------------
# Phase 2: segmented FFN. chunk_plan: list of (ncols, load_idx or None)
# --------------------------------------------------------------------------
def _phase2_nc(chunk_plan, nseg: int, C: int) -> bass.Bass:
    nc = bacc.Bacc("TRN2", target_bir_lowering=False, num_devices=NC)
    # hseg/oseg chunk-major: per chunk [p, (k c)] / [p, (mo c)] contiguous
    hseg = nc.dram_tensor("hseg", [128, DK * C], F16, kind="ExternalInput")
    gseg = nc.dram_tensor("gseg", [128, C], F16, kind="ExternalInput")
    # per segment: [p, (m k j)] for w1, [p, (m mo j)] for w2
    W = HK * DK * 128
    w1s = nc.dram_tensor("w1s", [nseg * 128, W], F16, kind="ExternalInput")
    w2s = nc.dram_tensor("w2s", [nseg * 128, W], F16, kind="ExternalInput")
    b1s = nc.dram_tensor("b1s", [128, nseg * HK], F32, kind="ExternalInput")
    b2s = nc.dram_tensor("b2s", [128, nseg * DK], F32, kind="ExternalInput")
    oseg = nc.dram_tensor("oseg", [128, DK * C], F16, kind="ExternalOutput")

    GW = MG * DK * 128  # columns per weight DMA group

    with tile.TileContext(nc) as tc:
        with (
            tc.tile_pool(name="const", bufs=1) as cpool,
            tc.tile_pool(name="w1p", bufs=3) as w1p,
            tc.tile_pool(name="w2p", bufs=3) as w2p,
            tc.tile_pool(name="hp", bufs=3) as hp,
            tc.tile_pool(name="hidp", bufs=3) as hidp,
            tc.tile_pool(name="op", bufs=3) as op,
            tc.tile_pool(name="hid_ps", bufs=3, space="PSUM") as hidps,
            tc.tile_pool(name="out_ps", bufs=1, space="PSUM") as outps,
        ):
            gseg_sb = cpool.tile([128, C], F16, tag="gseg")
            b1_sb = cpool.tile([128, nseg * HK], F32, tag="b1")
            b2_sb = cpool.tile([128, nseg * DK], F32, tag="b2")
            # all inputs ride the ACT HWDGE queue (no compute-dependent
            # waits there); outputs ride the SP queue so their semaphore
            # waits never block weight prefetch.
            nc.scalar.dma_start(b1_sb[:], b1s[:])
            nc.scalar.dma_start(b2_sb[:], b2s[:])

            # HAM warm-up spanning the input DMA wait (see phase 1)
            warm = cpool.tile([128, 128], F16, tag="warm")
            warm2 = cpool.tile([128, 512], F16, tag="warm2")
            nc.vector.memset(warm[:], 0.0)
            nc.vector.memset(warm2[:], 0.0)
            for _ in range(18):
                wps = hidps.tile([128, 512], F32, tag="hps")
                nc.tensor.matmul(wps[:], warm[:], warm2[:], start=True, stop=True)

            off = 0
            w1t = w2t = None
            li = -1
            first = True
            for ncols, load in chunk_plan:
                ht = hp.tile([128, DK * 512], F16, tag="h")
                nc.scalar.dma_start(
                    ht[:, : DK * ncols],
                    hseg[:, DK * off : DK * (off + ncols)],
                )
                if load is not None:
                    li = load
                    w1t = w1p.tile([128, W], F16, tag="w1")
                    w2t = w2p.tile([128, W], F16, tag="w2")
                    row = slice(li * 128, (li + 1) * 128)
                    for g in range(HK // MG):
                        gsl = slice(g * GW, (g + 1) * GW)
                        nc.scalar.dma_start(w1t[:, gsl], w1s[row, gsl])
                        nc.scalar.dma_start(w2t[:, gsl], w2s[row, gsl])
                if first:
                    # gates are not needed until the first chunk's combine
                    nc.scalar.dma_start(gseg_sb[:], gseg[:])
                    first = False
                opsum = outps.tile([128, DK, 512], F32, tag="opsum")
                for m in range(HK):
                    hps = hidps.tile([128, 512], F32, tag="hps")
                    for k in range(DK):
                        nc.tensor.matmul(
                            hps[:, :ncols],
                            w1t[:, m * 512 + k * 128 : m * 512 + (k + 1) * 128],
                            ht[:, k * ncols : (k + 1) * ncols],
                            start=(k == 0),
                            stop=(k == DK - 1),
                        )
                    hidt = hidp.tile([128, 512], F16, tag="hid")
                    nc.scalar.activation(
                        hidt[:, :ncols], hps[:, :ncols], AF.Gelu,
                        bias=b1_sb[:, li * HK + m : li * HK + m + 1],
                    )
                    for mo in range(DK):
                        nc.tensor.matmul(
                            opsum[:, mo, :ncols],
                            w2t[:, m * 512 + mo * 128 : m * 512 + (mo + 1) * 128],
                            hidt[:, :ncols],
                            start=(m == 0),
                            stop=(m == HK - 1),
                        )
                ot = op.tile([128, DK * 512], F16, tag="o")
                for mo in range(DK):
                    nc.vector.scalar_tensor_tensor(
                        ot[:, mo * ncols : (mo + 1) * ncols],
                        opsum[:, mo, :ncols],
                        b2_sb[:, li * DK + mo : li * DK + mo + 1],
                        gseg_sb[:, off : off + ncols],
                        ALU.add,
                        ALU.mult,
                    )
                nc.sync.dma_start(
                    oseg[:, DK * off : DK * (off + ncols)], ot[:, : DK * ncols]
                )
                off += ncols
    nc.compile()
    return nc


# --------------------------------------------------------------------------
# Segment packing: per-core-uniform pattern, single-expert segments
# --------------------------------------------------------------------------
def _pattern_for(q: int) -> list:
    """Descending segment sizes (in 128-col blocks) summing to q."""
    sizes = []
    while q > 0:
        if q <= 2:
            sizes.append(q)
            break
        if q == 3:
            sizes += [2, 1]
            break
        s = min(16, 1 << ((q // 2).bit_length() - 1))
        sizes.append(s)
        q -= s
    return sizes


def _plan_pack(block_need: dict):
    """block_need: {expert: nblocks}. Returns (pattern, claims) where claims
    is a list of (expert, size) in claim order, or (None, None)."""
    btot = sum(block_need.values())
    qmin = -(-btot // NC)
    for q in range(qmin, qmin + 9):
        pattern = _pattern_for(q)
        avail = {}
        for s in pattern:
            avail[s] = avail.get(s, 0) + NC
        claims = []
        ok = True
        for e, b in sorted(block_need.items(), key=lambda kv: -kv[1]):
            rem = b
            while rem > 0:
                cand = [s for s, c in avail.items() if c > 0]
                if not cand:
                    ok = False
                    break
                le = [s for s in cand if s <= rem]
                s = max(le) if le else min(cand)
                avail[s] -= 1
                claims.append((e, s))
                rem -= s
            if not ok:
                break
        if ok:
            return pattern, claims
    return None, None


def _run(nc, in_maps, label):
    trace = os.environ.get("KTRACE") == "1"
    res = run_bass_kernel_spmd(nc, in_maps, core_ids=list(range(NC)), trace=trace)
    if trace:
        last_stats[label] = {
            "exec_time_ns": res.exec_time_ns,
            "mean_exec_time_ns": res.mean_exec_time_ns,
            "trace": res.instructions_and_trace[1]
            if res.instructions_and_trace
            else None,
        }
    return res.results


def kernel(view0, view1, proj_w, proj_b, router_w, expert_keys, w1, b1, w2, b2):
    view0 = np.ascontiguousarray(view0, dtype=np.float32)
    view1 = np.ascontiguousarray(view1, dtype=np.float32)
    proj_w = np.asarray(proj_w, dtype=np.float32)
    proj_b = np.asarray(proj_b, dtype=np.float32)
    router_w = np.asarray(router_w, dtype=np.float32)
    keys = np.asarray(expert_keys, dtype=np.float32)
    w1 = np.asarray(w1, dtype=np.float32)
    b1 = np.asarray(b1, dtype=np.float32)
    w2 = np.asarray(w2, dtype=np.float32)
    b2 = np.asarray(b2, dtype=np.float32)

    # ---- Phase 1: h and d2 on device (token-parallel over 8 cores) ----
    xT_full = np.concatenate(
        [view0.reshape(N, D).T, view1.reshape(N, D).T], axis=1
    )  # [D, NT], column t = v*N + (b*T + tt)
    xT_d = xT_full.astype(np.float16)

    kT2 = np.ascontiguousarray(
        (-2.0 * keys.T).astype(np.float16).reshape(DK, 128, E).transpose(1, 0, 2)
    )
    kk1 = (keys * keys).sum(axis=1, dtype=np.float32).reshape(1, E)
    onc = np.ones((128, 1), np.float16)
    onr = np.ones((1, 512), np.float32)

    def pack_dd(w):  # [D, D] -> [128, (m k j)]
        return (
            w.astype(np.float16)
            .reshape(DK, 128, DK, 128)      # [k, p, m, j]
            .transpose(1, 2, 0, 3)          # [p, m, k, j]
            .reshape(128, DK * DK * 128)
        )

    in_maps1 = []
    for c in range(NC):
        v = (c * PC) // N  # cores 0-3 -> view 0, 4-7 -> view 1
        xc = xT_d[:, c * PC : (c + 1) * PC]  # [D, PC]
        xch = [
            xc[:, n * CH : (n + 1) * CH]
            .reshape(DK, 128, CH)
            .transpose(1, 0, 2)
            .reshape(128, DK * CH)
            for n in range(NCH)
        ]
        # r is computed directly from x: r = x @ (pw @ rw) + pb @ rw
        ws = [pack_dd(proj_w[v]), pack_dd(proj_w[v] @ router_w[v])]
        blobc = np.stack(
            [np.concatenate([ws[n], xch[n]], axis=1) for n in range(NCH)], axis=1
        )
        in_maps1.append(
            {
                "blob": np.ascontiguousarray(blobc),
                "pb": np.ascontiguousarray(proj_b[v].reshape(DK, 128).T),
                "prb": np.ascontiguousarray(
                    (proj_b[v] @ router_w[v]).reshape(DK, 128).T
                ),
                "kT2": kT2,
                "kk1": kk1,
                "onc": onc,
                "onr": onr,
            }
        )
    res1 = _run(_phase1_nc(), in_maps1, "phase1")

    hT_full = np.concatenate(
        [
            r["hT"].reshape(128, NCH, DK, CH).transpose(2, 0, 1, 3).reshape(D, PC)
            for r in res1
        ],
        axis=1,
    )  # [D, NT] fp16
    d2 = np.concatenate([r["d2T"] for r in res1], axis=1).T  # [NT, E] fp32

    # ---- Host repair: recompute borderline tokens exactly in fp32 ----
    logits0 = -np.sqrt(np.maximum(d2, 0.0), dtype=np.float32)
    part = np.partition(logits0, E - K - 1, axis=1)
    gap45 = part[:, E - K] - part[:, E - K - 1]
    risk = np.nonzero(gap45 < REPAIR_MARGIN)[0]
    last_stats["n_repaired"] = int(risk.size)
    if risk.size:
        x_all = np.concatenate([view0.reshape(N, D), view1.reshape(N, D)], axis=0)
        vsel = (risk >= N).astype(np.int64)
        kkr = kk1.reshape(E)
        for v in (0, 1):
            rt = risk[vsel == v]
            if rt.size == 0:
                continue
            hx = x_all[rt] @ proj_w[v] + proj_b[v]
            rx = hx @ router_w[v]
            d2[rt] = (
                (rx * rx).sum(axis=1, keepdims=True) - 2.0 * (rx @ keys.T) + kkr
            )

    # ---- Host routing: logits, top-4, softmax gates (fp32) ----
    logits = -np.sqrt(np.maximum(d2, 0.0), dtype=np.float32)
    topi = np.argsort(-logits, axis=1, kind="stable")[:, :K]  # [NT, K]
    topv = np.take_along_axis(logits, topi, axis=1)
    ex = np.exp(topv - topv[:, :1], dtype=np.float32)
    gates = ex / ex.sum(axis=1, keepdims=True, dtype=np.float32)

    # ---- Segment plan ----
    tok_e, g_e = {}, {}
    block_need = {}
    for e in range(E):
        sel_tok, sel_k = np.nonzero(topi == e)
        if sel_tok.size == 0:
            continue
        tok_e[e] = sel_tok
        g_e[e] = gates[sel_tok, sel_k]
        block_need[e] = -(-sel_tok.size // BLK)
    pattern, claims = _plan_pack(block_need)
    assert pattern is not None, "segment packing failed"
    # smallest segment first: the first chunk then only needs a tiny h
    # slice + first weight group, so real matmuls start ~7us earlier
    pattern = sorted(pattern)
    nseg = len(pattern)
    C = sum(pattern) * BLK  # columns per core
    chunk_plan = []
    for si, s in enumerate(pattern):
        cols = s * BLK
        firstc = True
        while cols > 0:
            n = min(512, cols)
            chunk_plan.append((n, si if firstc else None))
            firstc = False
            cols -= n
    last_stats["pattern"] = pattern
    last_stats["S"] = nseg
    last_stats["n_slots_real"] = len(claims)

    # assign claims to (core, seg_idx) instances, ordered by (position, core)
    inst = {}
    for si, s in enumerate(pattern):
        inst.setdefault(s, [])
        for c in range(NC):
            inst[s].append((c, si))
    ptrs = {s: 0 for s in inst}
    core_segs = [[None] * nseg for _ in range(NC)]
    epos = {e: 0 for e in tok_e}
    for e, s in claims:
        c, si = inst[s][ptrs[s]]
        ptrs[s] += 1
        lo = epos[e]
        hi = min(lo + s * BLK, tok_e[e].size)
        epos[e] = hi
        core_segs[c][si] = (e, tok_e[e][lo:hi], g_e[e][lo:hi])

    # ---- Phase 2 inputs ----
    hT16 = hT_full  # [D, NT] fp16
    W = HK * DK * 128
    w1_p, w2_p = {}, {}
    for e in tok_e:
        w1_p[e] = np.ascontiguousarray(
            w1[e].astype(np.float16)
            .reshape(DK, 128, HK, 128)    # [k, p, m, j]
            .transpose(1, 2, 0, 3)        # [p, m, k, j]
            .reshape(128, W)
        )
        w2_p[e] = np.ascontiguousarray(
            w2[e].astype(np.float16)
            .reshape(HK, 128, DK, 128)    # [m, p, mo, j]
            .transpose(1, 0, 2, 3)        # [p, m, mo, j]
            .reshape(128, W)
        )

    in_maps2 = []
    for c in range(NC):
        hsegf = np.zeros((128, DK * C), np.float16)
        grow = np.zeros((1, C), np.float16)
        w1c = np.zeros((nseg * 128, W), np.float16)
        w2c = np.zeros((nseg * 128, W), np.float16)
        b1c = np.zeros((128, nseg * HK), np.float32)
        b2c = np.zeros((128, nseg * DK), np.float32)
        off = 0
        for si, s in enumerate(pattern):
            seg = core_segs[c][si]
            cols = s * BLK
            if seg is not None:
                e, toks, gv = seg
                n = toks.size
                hcols = np.zeros((D, cols), np.float16)
                hcols[:, :n] = hT16[:, toks]
                grow[0, off : off + n] = gv.astype(np.float16)
                w1c[si * 128 : (si + 1) * 128] = w1_p[e]
                w2c[si * 128 : (si + 1) * 128] = w2_p[e]
                b1c[:, si * HK : (si + 1) * HK] = b1[e].reshape(HK, 128).T
                b2c[:, si * DK : (si + 1) * DK] = b2[e].reshape(DK, 128).T
            else:
                hcols = np.zeros((D, cols), np.float16)
            # pack this segment's chunks: per chunk [p, (k c)] contiguous
            co = 0
            while co < cols:
                n512 = min(512, cols - co)
                blkv = (
                    hcols[:, co : co + n512]
                    .reshape(DK, 128, n512)
                    .transpose(1, 0, 2)
                    .reshape(128, DK * n512)
                )
                hsegf[:, DK * (off + co) : DK * (off + co + n512)] = blkv
                co += n512
            off += cols
        in_maps2.append(
            {
                "hseg": hsegf,
                "gseg": np.ascontiguousarray(np.broadcast_to(grow, (128, C))),
                "w1s": w1c,
                "w2s": w2c,
                "b1s": b1c,
                "b2s": b2c,
            }
        )
    res2 = _run(_phase2_nc(chunk_plan, nseg, C), in_maps2, "phase2")

    # ---- Combine ----
    fusedT = np.zeros((D, NT), np.float32)
    for c in range(NC):
        o = res2[c]["oseg"].astype(np.float32)  # [128, DK*C] chunk-major
        oD = np.empty((D, C), np.float32)
        off = 0
        for ncols, _load in chunk_plan:
            blkv = (
                o[:, DK * off : DK * (off + ncols)]
                .reshape(128, DK, ncols)
                .transpose(1, 0, 2)
                .reshape(D, ncols)
            )
            oD[:, off : off + ncols] = blkv
            off += ncols
        off = 0
        for si, s in enumerate(pattern):
            seg = core_segs[c][si]
            cols = s * BLK
            if seg is not None and seg[1].size:
                toks = seg[1]
                fusedT[:, toks] += oD[:, off : off + toks.size]
            off += cols
    fused = (fusedT[:, :N] + fusedT[:, N:]).T  # [N, D]
    return np.ascontiguousarray(fused.reshape(B, T, D), dtype=np.float32)


# revision 42
# speedup vs baseline: 1.2592x; 1.2592x over previous
"""Trainium2 Bass kernel for nn_MoEElementFusion (moe_routing).

Strategy (8 NeuronCores, SPMD, two launches with host routing in between):
  Phase 1 (token-data-parallel): each core takes 1/8 of the 8192 (view,token)
  columns, streamed in 512-column chunks (chunk-major DRAM layouts so every
  DMA is contiguous with 4KB lines):
      h  = x @ proj_w + proj_b     (fp16 on the PE, psum fp32)
      r  = h @ router_w
      d2 = |r|^2 - 2 r.keys^T + |keys|^2
  h^T chunks are DMA'd out as they finish (overlapped with router compute).

  Host: tokens whose 4th/5th logit gap is under REPAIR_MARGIN get their d2
  row recomputed exactly in fp32; then logits, stable top-4, softmax gates.
  A segment plan packs each expert's selected tokens (128-col blocks) into a
  per-core-uniform pattern of segments (e.g. [16,8,4,2,2,1] blocks); each
  segment carries its own expert weights, so the padded column count per
  core is ~4224 instead of 9*512=4608 and weight DMA drops to ~6 loads.

  Phase 2 (compiled at runtime once the pattern is known): per segment, FFN
  in fp16 over 512/256/128-column chunks:
      out^T = (w2^T-mm(gelu(w1^T-mm(h^T) + b1)) + b2) * gates
  weights host-packed in the exact SBUF layout (contiguous, 4 m-tile groups
  interleaved w1/w2 for streaming), triple buffered; outputs stored fp16.

  Host combine: fused[:, tok] += out columns per segment; sum the two views.
"""

import math
import os

import numpy as np

import concourse.bass as bass
import concourse.bacc as bacc
import concourse.mybir as mybir
import concourse.tile as tile
from concourse.bass_utils import run_bass_kernel_spmd

# Problem dims (hardcoded per spec)
V, B, T, D, E, K = 2, 4, 1024, 512, 16, 4
H = 4 * D
N = B * T          # tokens per view
NT = V * N         # total (view, token) columns = 8192
NC = 8             # cores
PC = NT // NC      # phase-1 columns per core = 1024
BLK = 128          # phase-2 packing block (columns)

F32 = mybir.dt.float32
F16 = mybir.dt.float16
AF = mybir.ActivationFunctionType
ALU = mybir.AluOpType

DK = D // 128      # 4 k-tiles over D
HK = H // 128      # 16 k-tiles over H

CH = 512           # phase-1 column chunk
NCH = PC // CH     # 2 chunks

REPAIR_MARGIN = 0.02
MG = 2             # phase-2 weight DMA group: m-tiles per DMA

# Filled by kernel() for test harness introspection.
last_stats: dict = {}


# --------------------------------------------------------------------------
# Phase 1: h = x@pw + pb ; r = h@rw ; d2 = |r|^2 - 2 r.k + |k|^2
# --------------------------------------------------------------------------
def _phase1_nc() -> bass.Bass:
    nc = bacc.Bacc("TRN2", target_bir_lowering=False, num_devices=NC)
    # Two 1MB input blobs so the critical path is 1-2 large DMAs:
    #   blob n = [ w (2048 cols) | xT chunk n (2048 cols) ]  fp16
    # w packed [p, m*512 + k*128 + j] = w[k*128+p, m*128+j]; blob0 carries
    # pw, blob1 carries prw = proj_w @ router_w (r is computed directly
    # from x and never waits on the h activation writes).
    # xT chunk [p, k*CH+c] = x^T[k*128+p, n*CH+c]
    blob = nc.dram_tensor("blob", [128, NCH, 2 * DK * CH], F16, kind="ExternalInput")
    pb = nc.dram_tensor("pb", [128, DK], F32, kind="ExternalInput")
    prb = nc.dram_tensor("prb", [128, DK], F32, kind="ExternalInput")
    kT2 = nc.dram_tensor("kT2", [128, DK, E], F16, kind="ExternalInput")
    kk1 = nc.dram_tensor("kk1", [1, E], F32, kind="ExternalInput")
    onc = nc.dram_tensor("onc", [128, 1], F16, kind="ExternalInput")
    onr = nc.dram_tensor("onr", [1, 512], F32, kind="ExternalInput")
    hT = nc.dram_tensor("hT", [128, NCH, DK * CH], F16, kind="ExternalOutput")
    d2T = nc.dram_tensor("d2T", [E, PC], F32, kind="ExternalOutput")

    with tile.TileContext(nc) as tc:
        with (
            tc.tile_pool(name="const", bufs=1) as cpool,
            tc.tile_pool(name="act", bufs=1) as apool,
            tc.tile_pool(name="rt", bufs=2) as rtpool,
            tc.tile_pool(name="r2p", bufs=2) as r2pool,
            tc.tile_pool(name="ps", bufs=2, space="PSUM") as pspool,
            tc.tile_pool(name="ps_small", bufs=2, space="PSUM") as psmall,
        ):
            blob_sb = cpool.tile([128, NCH, 2 * DK * CH], F16, tag="blob")
            # inputs on the ACT HWDGE queue, critical path first:
            # blob0 = pw + x chunk 0, then the small constants, then blob1
            nc.scalar.dma_start(blob_sb[:, 0, :], blob[:, 0, :])
            pb_sb = cpool.tile([128, DK], F32, tag="pb")
            nc.scalar.dma_start(pb_sb[:], pb[:])
            prb_sb = cpool.tile([128, DK], F32, tag="prb")
            nc.scalar.dma_start(prb_sb[:], prb[:])
            nc.scalar.dma_start(blob_sb[:, 1, :], blob[:, 1, :])
            kT2_sb = cpool.tile([128, DK, E], F16, tag="kT2")
            nc.scalar.dma_start(kT2_sb[:], kT2[:])
            kk_sb = cpool.tile([1, E], F32, tag="kk")
            nc.scalar.dma_start(kk_sb[:], kk1[:])
            onc_sb = cpool.tile([128, 1], F16, tag="onc")
            nc.scalar.dma_start(onc_sb[:], onc[:])
            onr_sb = cpool.tile([1, 512], F32, tag="onr")
            nc.scalar.dma_start(onr_sb[:], onr[:])

            XO = DK * CH  # xT offset within a blob

            hT_sb = apool.tile([128, NCH, DK * CH], F16, tag="hT")
            rr_sb = apool.tile([1, PC], F32, tag="rr")
            d2_sb = apool.tile([E, PC], F32, tag="d2")

            # HAM warm-up: junk matmuls on zeroed SBUF spanning the input
            # DMA wait, so the real matmuls start at full clock.
            warm = cpool.tile([128, 128], F16, tag="warm")
            warm2 = cpool.tile([128, 512], F16, tag="warm2")
            nc.vector.memset(warm[:], 0.0)
            nc.vector.memset(warm2[:], 0.0)
            for _ in range(13):
                wps = pspool.tile([128, CH], F32, tag="ps")
                nc.tensor.matmul(wps[:], warm[:], warm2[:], start=True, stop=True)

            for n in range(NCH):
                sl = slice(n * CH, (n + 1) * CH)
                # h^T chunk
                for m in range(DK):
                    ps = pspool.tile([128, CH], F32, tag="ps")
                    for k in range(DK):
                        nc.tensor.matmul(
                            ps[:],
                            blob_sb[:, 0, m * 512 + k * 128 : m * 512 + (k + 1) * 128],
                            blob_sb[:, n, XO + k * CH : XO + (k + 1) * CH],
                            start=(k == 0),
                            stop=(k == DK - 1),
                        )
                    nc.scalar.activation(
                        hT_sb[:, n, m * CH : (m + 1) * CH], ps[:],
                        AF.Identity, bias=pb_sb[:, m : m + 1],
                    )
                    nc.sync.dma_start(
                        hT[:, n, m * CH : (m + 1) * CH],
                        hT_sb[:, n, m * CH : (m + 1) * CH],
                    )
                # r^T chunk (from x directly, via prw)
                rt = rtpool.tile([128, DK, CH], F16, tag="rt")
                for m in range(DK):
                    ps = pspool.tile([128, CH], F32, tag="ps")
                    for k in range(DK):
                        nc.tensor.matmul(
                            ps[:],
                            blob_sb[:, 1, m * 512 + k * 128 : m * 512 + (k + 1) * 128],
                            blob_sb[:, n, XO + k * CH : XO + (k + 1) * CH],
                            start=(k == 0),
                            stop=(k == DK - 1),
                        )
                    nc.scalar.activation(
                        rt[:, m, :], ps[:], AF.Identity, bias=prb_sb[:, m : m + 1]
                    )
                # rr = sum_d r^2
                r2_sb = r2pool.tile([128, DK, CH], F16, tag="r2")
                for k in range(DK):
                    nc.vector.tensor_mul(r2_sb[:, k, :], rt[:, k, :], rt[:, k, :])
                ps1 = psmall.tile([1, CH], F32, tag="ps1")
                for k in range(DK):
                    nc.tensor.matmul(
                        ps1[:], onc_sb[:], r2_sb[:, k, :],
                        start=(k == 0), stop=(k == DK - 1),
                    )
                nc.scalar.activation(rr_sb[:, sl], ps1[:], AF.Copy)
                # d2 = (-2 keys).r + |k|^2 + rr
                psA = psmall.tile([E, CH], F32, tag="psA")
                for k in range(DK):
                    nc.tensor.matmul(
                        psA[:], kT2_sb[:, k, :], rt[:, k, :],
                        start=(k == 0), stop=False,
                    )
                nc.tensor.matmul(psA[:], kk_sb[:], onr_sb[:], start=False, stop=False)
                nc.tensor.matmul(
                    psA[:], onr_sb[:, 0:E], rr_sb[:, sl], start=False, stop=True
                )
                nc.scalar.activation(d2_sb[:, sl], psA[:], AF.Copy)
                nc.sync.dma_start(d2T[:, sl], d2_sb[:, sl])
    nc.compile()
    return nc


# --------------------------------------------------------------------------
# Phase 2: segmented FFN. chunk_plan: list of (ncols, load_idx or None)
# --------------------------------------------------------------------------
def _phase2_nc(chunk_plan, nseg: int, C: int) -> bass.Bass:
    nc = bacc.Bacc("TRN2", target_bir_lowering=False, num_devices=NC)
    # hseg/oseg chunk-major: per chunk [p, (k c)] / [p, (mo c)] contiguous
    hseg = nc.dram_tensor("hseg", [128, DK * C], F16, kind="ExternalInput")
    gseg = nc.dram_tensor("gseg", [128, C], F16, kind="ExternalInput")
    # per segment: [p, (m k j)] for w1, [p, (m mo j)] for w2
    W = HK * DK * 128
    w1s = nc.dram_tensor("w1s", [nseg * 128, W], F16, kind="ExternalInput")
    w2s = nc.dram_tensor("w2s", [nseg * 128, W], F16, kind="ExternalInput")
    b1s = nc.dram_tensor("b1s", [128, nseg * HK], F32, kind="ExternalInput")
    b2s = nc.dram_tensor("b2s", [128, nseg * DK], F32, kind="ExternalInput")
    oseg = nc.dram_tensor("oseg", [128, DK * C], F16, kind="ExternalOutput")

    GW = MG * DK * 128  # columns per weight DMA group

    with tile.TileContext(nc) as tc:
        with (
            tc.tile_pool(name="const", bufs=1) as cpool,
            tc.tile_pool(name="w1p", bufs=3) as w1p,
            tc.tile_pool(name="w2p", bufs=3) as w2p,
            tc.tile_pool(name="hp", bufs=3) as hp,
            tc.tile_pool(name="hidp", bufs=3) as hidp,
            tc.tile_pool(name="op", bufs=3) as op,
            tc.tile_pool(name="hid_ps", bufs=3, space="PSUM") as hidps,
            tc.tile_pool(name="out_ps", bufs=1, space="PSUM") as outps,
        ):
            gseg_sb = cpool.tile([128, C], F16, tag="gseg")
            b1_sb = cpool.tile([128, nseg * HK], F32, tag="b1")
            b2_sb = cpool.tile([128, nseg * DK], F32, tag="b2")
            # all inputs ride the ACT HWDGE queue (no compute-dependent
            # waits there); outputs ride the SP queue so their semaphore
            # waits never block weight prefetch.
            nc.scalar.dma_start(b1_sb[:], b1s[:])
            nc.scalar.dma_start(b2_sb[:], b2s[:])

            # HAM warm-up spanning the input DMA wait (see phase 1)
            warm = cpool.tile([128, 128], F16, tag="warm")
            warm2 = cpool.tile([128, 512], F16, tag="warm2")
            nc.vector.memset(warm[:], 0.0)
            nc.vector.memset(warm2[:], 0.0)
            for _ in range(18):
                wps = hidps.tile([128, 512], F32, tag="hps")
                nc.tensor.matmul(wps[:], warm[:], warm2[:], start=True, stop=True)

            off = 0
            w1t = w2t = None
            li = -1
            first = True
            for ncols, load in chunk_plan:
                ht = hp.tile([128, DK * 512], F16, tag="h")
                nc.scalar.dma_start(
                    ht[:, : DK * ncols],
                    hseg[:, DK * off : DK * (off + ncols)],
                )
                if load is not None:
                    li = load
                    w1t = w1p.tile([128, W], F16, tag="w1")
                    w2t = w2p.tile([128, W], F16, tag="w2")
                    row = slice(li * 128, (li + 1) * 128)
                    for g in range(HK // MG):
                        gsl = slice(g * GW, (g + 1) * GW)
                        nc.scalar.dma_start(w1t[:, gsl], w1s[row, gsl])
                        nc.scalar.dma_start(w2t[:, gsl], w2s[row, gsl])
                if first:
                    # gates are not needed until the first chunk's combine
                    nc.scalar.dma_start(gseg_sb[:], gseg[:])
                    first = False
                opsum = outps.tile([128, DK, 512], F32, tag="opsum")
                for m in range(HK):
                    hps = hidps.tile([128, 512], F32, tag="hps")
                    for k in range(DK):
                        nc.tensor.matmul(
                            hps[:, :ncols],
                            w1t[:, m * 512 + k * 128 : m * 512 + (k + 1) * 128],
                            ht[:, k * ncols : (k + 1) * ncols],
                            start=(k == 0),
                            stop=(k == DK - 1),
                        )
                    hidt = hidp.tile([128, 512], F16, tag="hid")
                    nc.scalar.activation(
                        hidt[:, :ncols], hps[:, :ncols], AF.Gelu,
                        bias=b1_sb[:, li * HK + m : li * HK + m + 1],
                    )
                    for mo in range(DK):
                        nc.tensor.matmul(
                            opsum[:, mo, :ncols],
                            w2t[:, m * 512 + mo * 128 : m * 512 + (mo + 1) * 128],
                            hidt[:, :ncols],
                            start=(m == 0),
                            stop=(m == HK - 1),
                        )
                ot = op.tile([128, DK * 512], F16, tag="o")
                for mo in range(DK):
                    nc.vector.scalar_tensor_tensor(
                        ot[:, mo * ncols : (mo + 1) * ncols],
                        opsum[:, mo, :ncols],
                        b2_sb[:, li * DK + mo : li * DK + mo + 1],
                        gseg_sb[:, off : off + ncols],
                        ALU.add,
                        ALU.mult,
                    )
                nc.sync.dma_start(
                    oseg[:, DK * off : DK * (off + ncols)], ot[:, : DK * ncols]
                )
                off += ncols
    nc.compile()
    return nc


# --------------------------------------------------------------------------
# Segment packing: per-core-uniform pattern, single-expert segments
# --------------------------------------------------------------------------
def _pattern_for(q: int) -> list:
    """Descending segment sizes (in 128-col blocks) summing to q."""
    sizes = []
    while q > 0:
        if q <= 2:
            sizes.append(q)
            break
        if q == 3:
            sizes += [2, 1]
            break
        s = min(16, 1 << ((q // 2).bit_length() - 1))
        sizes.append(s)
        q -= s
    return sizes


def _plan_pack(block_need: dict):
    """block_need: {expert: nblocks}. Returns (pattern, claims) where claims
    is a list of (expert, size) in claim order, or (None, None)."""
    btot = sum(block_need.values())
    qmin = -(-btot // NC)
    for q in range(qmin, qmin + 9):
        pattern = _pattern_for(q)
        avail = {}
        for s in pattern:
            avail[s] = avail.get(s, 0) + NC
        claims = []
        ok = True
        for e, b in sorted(block_need.items(), key=lambda kv: -kv[1]):
            rem = b
            while rem > 0:
                cand = [s for s, c in avail.items() if c > 0]
                if not cand:
                    ok = False
                    break
                le = [s for s in cand if s <= rem]
                s = max(le) if le else min(cand)
                avail[s] -= 1
                claims.append((e, s))
                rem -= s
            if not ok:
                break
        if ok:
            return pattern, claims
    return None, None


def _run(nc, in_maps, label):
    trace = os.environ.get("KTRACE") == "1"
    res = run_bass_kernel_spmd(nc, in_maps, core_ids=list(range(NC)), trace=trace)
    if trace:
        last_stats[label] = {
            "exec_time_ns": res.exec_time_ns,
            "mean_exec_time_ns": res.mean_exec_time_ns,
            "trace": res.instructions_and_trace[1]
            if res.instructions_and_trace
            else None,
        }
    return res.results


def kernel(view0, view1, proj_w, proj_b, router_w, expert_keys, w1, b1, w2, b2):
    view0 = np.ascontiguousarray(view0, dtype=np.float32)
    view1 = np.ascontiguousarray(view1, dtype=np.float32)
    proj_w = np.asarray(proj_w, dtype=np.float32)
    proj_b = np.asarray(proj_b, dtype=np.float32)
    router_w = np.asarray(router_w, dtype=np.float32)
    keys = np.asarray(expert_keys, dtype=np.float32)
    w1 = np.asarray(w1, dtype=np.float32)
    b1 = np.asarray(b1, dtype=np.float32)
    w2 = np.asarray(w2, dtype=np.float32)
    b2 = np.asarray(b2, dtype=np.float32)

    # ---- Phase 1: h and d2 on device (token-parallel over 8 cores) ----
    xT_full = np.concatenate(
        [view0.reshape(N, D).T, view1.reshape(N, D).T], axis=1
    )  # [D, NT], column t = v*N + (b*T + tt)
    xT_d = xT_full.astype(np.float16)

    kT2 = np.ascontiguousarray(
        (-2.0 * keys.T).astype(np.float16).reshape(DK, 128, E).transpose(1, 0, 2)
    )
    kk1 = (keys * keys).sum(axis=1, dtype=np.float32).reshape(1, E)
    onc = np.ones((128, 1), np.float16)
    onr = np.ones((1, 512), np.float32)

    def pack_dd(w):  # [D, D] -> [128, (m k j)]
        return (
            w.astype(np.float16)
            .reshape(DK, 128, DK, 128)      # [k, p, m, j]
            .transpose(1, 2, 0, 3)          # [p, m, k, j]
            .reshape(128, DK * DK * 128)
        )

    in_maps1 = []
    for c in range(NC):
        v = (c * PC) // N  # cores 0-3 -> view 0, 4-7 -> view 1
        xc = xT_d[:, c * PC : (c + 1) * PC]  # [D, PC]
        xch = [
            xc[:, n * CH : (n + 1) * CH]
            .reshape(DK, 128, CH)
            .transpose(1, 0, 2)
            .reshape(128, DK * CH)
            for n in range(NCH)
        ]
        # r is computed directly from x: r = x @ (pw @ rw) + pb @ rw
        ws = [pack_dd(proj_w[v]), pack_dd(proj_w[v] @ router_w[v])]
        blobc = np.stack(
            [np.concatenate([ws[n], xch[n]], axis=1) for n in range(NCH)], axis=1
        )
        in_maps1.append(
            {
                "blob": np.ascontiguousarray(blobc),
                "pb": np.ascontiguousarray(proj_b[v].reshape(DK, 128).T),
                "prb": np.ascontiguousarray(
                    (proj_b[v] @ router_w[v]).reshape(DK, 128).T
                ),
                "kT2": kT2,
                "kk1": kk1,
                "onc": onc,
                "onr": onr,
            }
        )
    res1 = _run(_phase1_nc(), in_maps1, "phase1")

    hT_full = np.concatenate(
        [
            r["hT"].reshape(128, NCH, DK, CH).transpose(2, 0, 1, 3).reshape(D, PC)
            for r in res1
        ],
        axis=1,
    )  # [D, NT] fp16
    d2 = np.concatenate([r["d2T"] for r in res1], axis=1).T  # [NT, E] fp32

    # ---- Host repair: recompute borderline tokens exactly in fp32 ----
    logits0 = -np.sqrt(np.maximum(d2, 0.0), dtype=np.float32)
    part = np.partition(logits0, E - K - 1, axis=1)
    gap45 = part[:, E - K] - part[:, E - K - 1]
    risk = np.nonzero(gap45 < REPAIR_MARGIN)[0]
    last_stats["n_repaired"] = int(risk.size)
    if risk.size:
        x_all = np.concatenate([view0.reshape(N, D), view1.reshape(N, D)], axis=0)
        vsel = (risk >= N).astype(np.int64)
        kkr = kk1.reshape(E)
        for v in (0, 1):
            rt = risk[vsel == v]
            if rt.size == 0:
                continue
            hx = x_all[rt] @ proj_w[v] + proj_b[v]
            rx = hx @ router_w[v]
            d2[rt] = (
                (rx * rx).sum(axis=1, keepdims=True) - 2.0 * (rx @ keys.T) + kkr
            )

    # ---- Host routing: logits, top-4, softmax gates (fp32) ----
    logits = -np.sqrt(np.maximum(d2, 0.0), dtype=np.float32)
    topi = np.argsort(-logits, axis=1, kind="stable")[:, :K]  # [NT, K]
    topv = np.take_along_axis(logits, topi, axis=1)
    ex = np.exp(topv - topv[:, :1], dtype=np.float32)
    gates = ex / ex.sum(axis=1, keepdims=True, dtype=np.float32)

    # ---- Segment plan ----
    tok_e, g_e = {}, {}
    block_need = {}
    for e in range(E):
        sel_tok, sel_k = np.nonzero(topi == e)
        if sel_tok.size == 0:
            continue
        tok_e[e] = sel_tok
        g_e[e] = gates[sel_tok, sel_k]
        block_need[e] = -(-sel_tok.size // BLK)
    pattern, claims = _plan_pack(block_need)
    assert pattern is not None, "segment packing failed"
    # largest segment first: its long compute absorbs the prefetch of all
    # later segments' weights (small-first measured 8-9us stalls per
    # segment transition while the 24MB weight stream caught up)
    pattern = sorted(pattern, reverse=True)
    nseg = len(pattern)
    C = sum(pattern) * BLK  # columns per core
    chunk_plan = []
    for si, s in enumerate(pattern):
        cols = s * BLK
        firstc = True
        while cols > 0:
            n = min(512, cols)
            chunk_plan.append((n, si if firstc else None))
            firstc = False
            cols -= n
    last_stats["pattern"] = pattern
    last_stats["S"] = nseg
    last_stats["n_slots_real"] = len(claims)

    # assign claims to (core, seg_idx) instances, ordered by (position, core)
    inst = {}
    for si, s in enumerate(pattern):
        inst.setdefault(s, [])
        for c in range(NC):
            inst[s].append((c, si))
    ptrs = {s: 0 for s in inst}
    core_segs = [[None] * nseg for _ in range(NC)]
    epos = {e: 0 for e in tok_e}
    for e, s in claims:
        c, si = inst[s][ptrs[s]]
        ptrs[s] += 1
        lo = epos[e]
        hi = min(lo + s * BLK, tok_e[e].size)
        epos[e] = hi
        core_segs[c][si] = (e, tok_e[e][lo:hi], g_e[e][lo:hi])

    # ---- Phase 2 inputs ----
    hT16 = hT_full  # [D, NT] fp16
    W = HK * DK * 128
    w1_p, w2_p = {}, {}
    for e in tok_e:
        w1_p[e] = np.ascontiguousarray(
            w1[e].astype(np.float16)
            .reshape(DK, 128, HK, 128)    # [k, p, m, j]
            .transpose(1, 2, 0, 3)        # [p, m, k, j]
            .reshape(128, W)
        )
        w2_p[e] = np.ascontiguousarray(
            w2[e].astype(np.float16)
            .reshape(HK, 128, DK, 128)    # [m, p, mo, j]
            .transpose(1, 0, 2, 3)        # [p, m, mo, j]
            .reshape(128, W)
        )

    in_maps2 = []
    for c in range(NC):
        hsegf = np.zeros((128, DK * C), np.float16)
        grow = np.zeros((1, C), np.float16)
        w1c = np.zeros((nseg * 128, W), np.float16)
        w2c = np.zeros((nseg * 128, W), np.float16)
        b1c = np.zeros((128, nseg * HK), np.float32)
        b2c = np.zeros((128, nseg * DK), np.float32)
        off = 0
        for si, s in enumerate(pattern):
            seg = core_segs[c][si]
            cols = s * BLK
            if seg is not None:
                e, toks, gv = seg
                n = toks.size
                hcols = np.zeros((D, cols), np.float16)
                hcols[:, :n] = hT16[:, toks]
                grow[0, off : off + n] = gv.astype(np.float16)
                w1c[si * 128 : (si + 1) * 128] = w1_p[e]
                w2c[si * 128 : (si + 1) * 128] = w2_p[e]
                b1c[:, si * HK : (si + 1) * HK] = b1[e].reshape(HK, 128).T
                b2c[:, si * DK : (si + 1) * DK] = b2[e].reshape(DK, 128).T
            else:
                hcols = np.zeros((D, cols), np.float16)
            # pack this segment's chunks: per chunk [p, (k c)] contiguous
            co = 0
            while co < cols:
                n512 = min(512, cols - co)
                blkv = (
                    hcols[:, co : co + n512]
                    .reshape(DK, 128, n512)
                    .transpose(1, 0, 2)
                    .reshape(128, DK * n512)
                )
                hsegf[:, DK * (off + co) : DK * (off + co + n512)] = blkv
                co += n512
            off += cols
        in_maps2.append(
            {
                "hseg": hsegf,
                "gseg": np.ascontiguousarray(np.broadcast_to(grow, (128, C))),
                "w1s": w1c,
                "w2s": w2c,
                "b1s": b1c,
                "b2s": b2c,
            }
        )
    res2 = _run(_phase2_nc(chunk_plan, nseg, C), in_maps2, "phase2")

    # ---- Combine ----
    fusedT = np.zeros((D, NT), np.float32)
    for c in range(NC):
        o = res2[c]["oseg"].astype(np.float32)  # [128, DK*C] chunk-major
        oD = np.empty((D, C), np.float32)
        off = 0
        for ncols, _load in chunk_plan:
            blkv = (
                o[:, DK * off : DK * (off + ncols)]
                .reshape(128, DK, ncols)
                .transpose(1, 0, 2)
                .reshape(D, ncols)
            )
            oD[:, off : off + ncols] = blkv
            off += ncols
        off = 0
        for si, s in enumerate(pattern):
            seg = core_segs[c][si]
            cols = s * BLK
            if seg is not None and seg[1].size:
                toks = seg[1]
                fusedT[:, toks] += oD[:, off : off + toks.size]
            off += cols
    fused = (fusedT[:, :N] + fusedT[:, N:]).T  # [N, D]
    return np.ascontiguousarray(fused.reshape(B, T, D), dtype=np.float32)
